# revision 14
# baseline (speedup 1.0000x reference)
"""Trainium2 Bass/Tile kernel for the sparse-attention nn.Module (fp16 rewrite).

Math (per batch b):
    Q = Wq @ x1 + bq            [32, N]     (N = 128*128 = 16384)
    K = Wk @ x1 + bk            [32, N]
    V = Wv @ x  + bv            [192, N]
    Qn = Q / ||Q||_col, Kn = K / ||K||_col
    tailor[n] = 1 / (N + Qn[:,n].(ksum+EPS)),  ksum = sum_n Kn[:,n]
    out[c,n]  = gamma * tailor[n] * (vsum[c] + sum_m Qn[m,n] matrix[m,c])
    matrix = Kn V^T, vsum = V.sum(n)

Key restructurings vs the fp32r baseline (235 us):
  * fp16 I/O end-to-end: x1, x^T staged fp16 (halves input DMA), output
    staged fp16 position-major and unswizzled + gamma-scaled on host.
    Simulated end-to-end rel err of this exact pipeline: 1.1e-3.
  * V is never materialized: matrix = (Kn^T X^T) Wv^T + ksum (x) bv and
    vsum = Wv xsum + N bv. Phase 1 accumulates S = [Kn|1]^T [X^T|1]
    (33 x 193, one accumulating matmul across all 128 sub-chunks); a tiny
    once-per-batch phase 1.9 contracts S with Wv^T. Kills the V matmuls
    (2 of 5 per sub-chunk) and the big V psum->sbuf copy.
  * Qn never materialized: 1/||Q|| folds into the phase-2 per-position
    scalars (qs = Q * (tailor_n * rn_q), tailor via raw-Q dot).
  * Norm pipeline batched per 2048-position chunk (1 square / 1 reduce /
    1 sqrt / 1 reciprocal / 1 scaled K-copy / 1 Q-copy) instead of
    per-128-position ops -- ACT/DVE instruction overhead was ~30% of the
    baseline's wall clock.
  * Phase 2 emits out in [position, channel] layout: one [128,384] matmul
    per 2 sub-chunks with a block-diagonal [66,384] rhs built from
    matrix'/vsum', stationary = transposed (qs-pair). One psum tile and
    one copy per 4 sub-chunks; out DMA'd position-major.

Distribution: data-parallel over batch (B == 8 == n_cores), no collectives.
"""

import numpy as np

import concourse.bass as bass
import concourse.mybir as mybir
import concourse.tile as tile
from concourse import bacc
from concourse.bass_utils import run_bass_kernel_spmd

F16 = mybir.dt.float16
F32 = mybir.dt.float32
AX = mybir.AxisListType
AF = mybir.ActivationFunctionType
ALU = mybir.AluOpType

N_CORES = 8
B, C, H, W = 8, 192, 128, 128
CQ = 32
N = H * W              # 16384
EPS = 1e-6

SUB = 128              # positions per matmul sub-chunk
NSUB = N // SUB        # 128
CHUNK = 2048           # positions per phase-1 chunk
NCHUNK = N // CHUNK    # 8
SPC = CHUNK // SUB     # 16 sub-chunks per chunk
XTB = 4                # sub-chunks per xt DMA block
GRP = 16               # sub-chunks per phase-2 group
NGRP = NSUB // GRP     # 8


def _view(ap, offset_elems, pattern):
    """Raw AP view: pattern is [[step, num], ...] in elements."""
    return bass.AP(tensor=ap.tensor, offset=ap.offset + offset_elems,
                   ap=pattern)


def build_program():
    nc = bacc.Bacc("TRN2", target_bir_lowering=False, debug=False,
                   num_devices=N_CORES)

    x1 = nc.dram_tensor("x1", [C, N], F16, kind="ExternalInput").ap()
    xt = nc.dram_tensor("xt", [128, NSUB * C], F16, kind="ExternalInput").ap()
    wqk1 = nc.dram_tensor("wqk1", [128, 2 * CQ], F16, kind="ExternalInput").ap()
    wqk2 = nc.dram_tensor("wqk2", [65, 2 * CQ], F16, kind="ExternalInput").ap()
    wv1 = nc.dram_tensor("wv1", [128, C], F16, kind="ExternalInput").ap()
    wv2 = nc.dram_tensor("wv2", [65, C], F16, kind="ExternalInput").ap()
    ident_d = nc.dram_tensor("ident_d", [128, 128], F16,
                             kind="ExternalInput").ap()
    osw = nc.dram_tensor("osw", [128, NSUB * C], F16,
                         kind="ExternalOutput").ap()

    with tile.TileContext(nc) as tc, nc.allow_low_precision(
            reason="fp16 pipeline validated end-to-end on host: rel err 1.1e-3"):
        with tc.tile_pool(name="singles", bufs=1) as sg:
            w_qk1 = sg.tile([128, 2 * CQ], F16)
            nc.sync.dma_start(out=w_qk1, in_=wqk1)
            w_qk2 = sg.tile([65, 2 * CQ], F16)
            nc.sync.dma_start(out=w_qk2, in_=wqk2)
            w_v1 = sg.tile([128, C], F16)
            nc.sync.dma_start(out=w_v1, in_=wv1)
            w_v2 = sg.tile([65, C], F16)
            nc.sync.dma_start(out=w_v2, in_=wv2)
            ident = sg.tile([128, 128], F16)
            nc.sync.dma_start(out=ident, in_=ident_d)

            ones_col = sg.tile([1, 128], F16)
            nc.vector.memset(ones_col, 1.0)

            # x1 rows 128:192 + a ones row (row 64) folding the bias in.
            x1b_t = [sg.tile([65, CHUNK], F16, tag=f"x1b{i}", name=f"x1b{i}")
                     for i in range(2)]
            for t in x1b_t:
                nc.vector.memset(t[64:65], 1.0)
            # X^T tiles [128, XTB, 193]; col 192 = ones (ksum column).
            xt_t = [sg.tile([128, XTB, C + 1], F16, tag=f"xt{i}",
                            name=f"xt{i}") for i in range(2 * SPC // XTB)]
            for t in xt_t:
                nc.vector.memset(t[:, :, C:C + 1], 1.0)
            # Kn_aug [128, SPC, 33]; col 32 = ones (xsum row).
            kn_t = [sg.tile([128, SPC, CQ + 1], F16, tag=f"kn{i}",
                            name=f"kn{i}") for i in range(2)]
            for t in kn_t:
                nc.vector.memset(t[:, :, CQ:CQ + 1], 1.0)

            qbuf = sg.tile([128, NSUB, CQ], F16)     # raw Q, pos-major
            rn_all = sg.tile([128, 2 * NSUB], F16)   # 1/||Q||,1/||K|| interlv
            kse_sb = sg.tile([128, CQ], F16)         # (ksum+EPS)/N bcast
            # mt'_aug duplicated on partitions 0:33 and 64:97 for row-tiled
            # concurrent phase-2 matmuls; rows 33:64 zero.
            mt2 = sg.tile([97, C], F16)
            nc.vector.memset(mt2, 0.0)
            s_sb = sg.tile([33, C + 1], F16)
            stl = sg.tile([128, CQ + 1], F16)
            sth = sg.tile([65, CQ + 1], F16)
            mt_sb = sg.tile([33, C], F16)
            mtt_l = sg.tile([128, 33], F16)
            mtt_h = sg.tile([64, 33], F16)
            # qs pair tiles [128, 8 pairs, 97]: subA cols 0:33, subB 64:97,
            # gap cols 33:64 zeroed once (transposed into zero lhsT rows).
            qs_t = [sg.tile([128, GRP // 2, 97], F16, tag=f"qs{i}",
                            name=f"qs{i}") for i in range(2)]
            for t in qs_t:
                nc.vector.memset(t[:, :, 33:64], 0.0)

            # ---------------- phase 1 ----------------
            with tc.tile_pool(name="sps", bufs=1, space="PSUM") as sps_pool, \
                 tc.tile_pool(name="xin", bufs=2) as xin, \
                 tc.tile_pool(name="qkps", bufs=2, space="PSUM") as qkps, \
                 tc.tile_pool(name="p1s", bufs=2) as p1s:
                s_ps = sps_pool.tile([33, C + 1], F32)
                xts_of = {}

                def emit_s_mms(ci):
                    # accumulate S += Kn_aug^T @ [X^T | 1] for chunk ci
                    kn = kn_t[ci % 2]
                    for si in range(SPC):
                        sub = ci * SPC + si
                        nc.tensor.matmul(
                            s_ps, lhsT=kn[:, si, :],
                            rhs=xts_of[ci][si // XTB][:, si % XTB, :],
                            start=(sub == 0), stop=(sub == NSUB - 1))

                for ci in range(NCHUNK):
                    n0 = ci * CHUNK
                    x1a = xin.tile([128, CHUNK], F16, tag="x1a")
                    nc.sync.dma_start(out=x1a, in_=x1[0:128, n0:n0 + CHUNK])
                    x1b = x1b_t[ci % 2]
                    nc.gpsimd.dma_start(out=x1b[0:64],
                                        in_=x1[128:C, n0:n0 + CHUNK])
                    xts = []
                    for q in range(SPC // XTB):
                        t = xt_t[(ci % 2) * (SPC // XTB) + q]
                        s0 = ci * SPC + q * XTB
                        nc.sync.dma_start(
                            out=t[:, :, 0:C],
                            in_=xt[:, s0 * C:(s0 + XTB) * C].rearrange(
                                "p (k c) -> p k c", c=C))
                        xts.append(t)
                    xts_of[ci] = xts

                    qk_ps = qkps.tile([128, SPC * 2 * CQ], F32, tag="qk")
                    for si in range(SPC):
                        cs = slice(si * 2 * CQ, (si + 1) * 2 * CQ)
                        ps = slice(si * SUB, (si + 1) * SUB)
                        nc.tensor.matmul(qk_ps[:, cs], lhsT=x1a[:, ps],
                                         rhs=w_qk1, start=True, stop=False)
                        nc.tensor.matmul(qk_ps[:, cs], lhsT=x1b[:, ps],
                                         rhs=w_qk2, start=False, stop=True)

                    # batched norm pipeline for the whole chunk
                    sq = p1s.tile([128, SPC * 2 * CQ], F16, tag="sq")
                    nc.scalar.activation(out=sq, in_=qk_ps, func=AF.Square)
                    ss = p1s.tile([128, 2 * SPC], F16, tag="ss")
                    nc.vector.reduce_sum(
                        ss, sq.rearrange("p (s c) -> p s c", c=CQ), axis=AX.X)
                    nrm = p1s.tile([128, 2 * SPC], F16, tag="nrm")
                    nc.scalar.sqrt(nrm, ss)
                    rn = rn_all[:, ci * 2 * SPC:(ci + 1) * 2 * SPC]
                    nc.vector.reciprocal(rn, nrm)

                    # raw Q -> qbuf (ACT), Kn = K * rn_k -> kn tile (DVE)
                    qv = _view(qk_ps, 0,
                               [qk_ps.ap[0], [2 * CQ, SPC], [1, CQ]])
                    nc.scalar.copy(qbuf[:, ci * SPC:(ci + 1) * SPC, :], qv)
                    kv = _view(qk_ps, CQ,
                               [qk_ps.ap[0], [2 * CQ, SPC], [1, CQ]])
                    rkv = _view(rn_all, ci * 2 * SPC + 1,
                                [rn_all.ap[0], [2, SPC], [0, CQ]])
                    kn = kn_t[ci % 2]
                    nc.vector.tensor_mul(kn[:, :, 0:CQ], kv, rkv)

                    # software pipelining: the PREVIOUS chunk's S-matmuls go
                    # behind this chunk's qk-matmuls in the PE queue, so the
                    # PE never stalls on the current chunk's norm chain.
                    if ci > 0:
                        emit_s_mms(ci - 1)
                emit_s_mms(NCHUNK - 1)

                # ---- phase 1.9: S -> matrix'/vsum', ksum -> kse ----
                nc.vector.tensor_copy(s_sb, s_ps)

            with tc.tile_pool(name="p19", bufs=1, space="PSUM") as p19:
                stl_ps = p19.tile([128, CQ + 1], F16, tag="stl")
                nc.tensor.transpose(stl_ps, s_sb[:, 0:128],
                                    ident[0:33, 0:33])
                sth_ps = p19.tile([65, CQ + 1], F16, tag="sth")
                nc.tensor.transpose(sth_ps, s_sb[:, 128:C + 1],
                                    ident[0:33, 0:33])
                nc.vector.tensor_copy(stl, stl_ps)
                nc.vector.tensor_copy(sth, sth_ps)

                mt_ps = p19.tile([33, C], F32, tag="mt")
                nc.tensor.matmul(mt_ps, lhsT=stl, rhs=w_v1,
                                 start=True, stop=False)
                nc.tensor.matmul(mt_ps, lhsT=sth, rhs=w_v2,
                                 start=False, stop=True)
                # mt' = [matrix | vsum] / N   (gamma applied on host)
                nc.scalar.activation(out=mt_sb, in_=mt_ps, func=AF.Copy,
                                     scale=1.0 / N)
                # duplicate mt' onto partitions 0:33 and 64:97 via PE
                # transposes (engines cannot shift lanes); rows 33:64 stay 0.
                mtt_l_ps = p19.tile([128, 33], F16, tag="mtl", name="mtl")
                nc.tensor.transpose(mtt_l_ps, mt_sb[:, 0:128],
                                    ident[0:33, 0:33])
                mtt_h_ps = p19.tile([64, 33], F16, tag="mth", name="mth")
                nc.tensor.transpose(mtt_h_ps, mt_sb[:, 128:C],
                                    ident[0:33, 0:33])
                nc.vector.tensor_copy(mtt_l, mtt_l_ps)
                nc.vector.tensor_copy(mtt_h, mtt_h_ps)
                mt2_ps = p19.tile([97, C], F16, tag="mt2")
                nc.tensor.transpose(mt2_ps[0:33, 0:128], mtt_l, ident)
                nc.tensor.transpose(mt2_ps[0:33, 128:C], mtt_h,
                                    ident[0:64, 0:64])
                nc.tensor.transpose(mt2_ps[64:97, 0:128], mtt_l, ident,
                                    tile_position=(0, 64))
                nc.tensor.transpose(mt2_ps[64:97, 128:C], mtt_h,
                                    ident[0:64, 0:64], tile_position=(0, 64))
                nc.vector.tensor_copy(mt2[0:33], mt2_ps[0:33])
                nc.vector.tensor_copy(mt2[64:97], mt2_ps[64:97])

                # kse = (ksum + EPS) / N, broadcast to 128 partitions via PE
                kse_row = sg.tile([1, CQ], F16)
                nc.vector.tensor_scalar(
                    out=kse_row, in0=sth[64:65, 0:CQ], scalar1=EPS,
                    scalar2=1.0 / N, op0=ALU.add, op1=ALU.mult)
                kb_ps = p19.tile([128, CQ], F32, tag="kb")
                nc.tensor.matmul(kb_ps, lhsT=ones_col, rhs=kse_row)
                nc.vector.tensor_copy(kse_sb, kb_ps)

            # ---------------- phase 1.5 + 2 ----------------
            with tc.tile_pool(name="p2s", bufs=2) as p2s, \
                 tc.tile_pool(name="trps", bufs=2, space="PSUM") as trps, \
                 tc.tile_pool(name="ops", bufs=3, space="PSUM") as ops_pool, \
                 tc.tile_pool(name="obs", bufs=3) as obs:
                kse_b = _view(kse_sb, 0, [kse_sb.ap[0], [0, GRP], [1, CQ]])
                for g in range(NGRP):
                    qb_g = qbuf[:, g * GRP:(g + 1) * GRP, :]
                    prod = p2s.tile([128, GRP, CQ], F16, tag="prod")
                    nc.gpsimd.tensor_mul(prod, qb_g, kse_b)
                    dot = p2s.tile([128, GRP], F16, tag="dot")
                    nc.vector.reduce_sum(dot, prod, axis=AX.X)
                    rq = _view(rn_all, g * 2 * GRP,
                               [rn_all.ap[0], [2, GRP]])
                    tg = p2s.tile([128, GRP], F16, tag="tg")
                    nc.vector.tensor_mul(tg, dot, rq)
                    nc.vector.tensor_scalar_add(tg, tg, 1.0)
                    nc.vector.reciprocal(tg, tg)
                    s2 = p2s.tile([128, GRP], F16, tag="s2")
                    nc.vector.tensor_mul(s2, tg, rq)

                    # qs pair tile: subA (even subs) cols 0:33,
                    # subB (odd subs) cols 64:97; qs = [Q * s2 | tg]
                    qs = qs_t[g % 2]
                    q0 = g * GRP * CQ
                    for par, c0 in ((0, 0), (1, 64)):
                        qv = _view(qbuf, q0 + par * CQ,
                                   [qbuf.ap[0], [2 * CQ, GRP // 2], [1, CQ]])
                        sv = _view(s2, par, [s2.ap[0], [2, GRP // 2],
                                             [0, CQ]])
                        nc.vector.tensor_mul(
                            _view(qs, c0, [qs.ap[0], [97, GRP // 2],
                                           [1, CQ]]), qv, sv)
                        nc.vector.tensor_copy(
                            _view(qs, c0 + CQ, [qs.ap[0], [97, GRP // 2],
                                                [1, 1]]),
                            _view(tg, par, [tg.ap[0], [2, GRP // 2],
                                            [1, 1]]))

                    for h in range(GRP // (2 * XTB)):     # 2 stage batches
                        tr_ps = trps.tile([97, XTB, 128], F16, tag="tr")
                        for j in range(XTB):              # 4 transposed pairs
                            pr = 4 * h + j
                            nc.tensor.transpose(
                                tr_ps[:, j, :], qs[:, pr, :], ident)
                        stage = p2s.tile([97, XTB, 128], F16, tag="stage")
                        if h % 2 == 0:
                            nc.vector.tensor_copy(stage, tr_ps)
                        else:
                            nc.scalar.copy(stage, tr_ps)
                        # per pair: two row-tiled concurrent matmuls
                        # (rows 0:33 and 64:97), outputs in separate banks
                        for jj in range(XTB // 2):
                            o_ps = ops_pool.tile([128, 1024], F32, tag="ops")
                            for j in (2 * jj, 2 * jj + 1):
                                c0 = (j % 2) * C
                                nc.tensor.matmul(
                                    o_ps[:, c0:c0 + C],
                                    lhsT=stage[0:33, j, :], rhs=mt2[0:33])
                                nc.tensor.matmul(
                                    o_ps[:, 512 + c0:512 + c0 + C],
                                    lhsT=stage[64:97, j, :], rhs=mt2[64:97],
                                    tile_position=(64, 0))
                            ob = obs.tile([128, 4 * C], F16, tag="ob")
                            # position order A0 B0 A1 B1 ->
                            # cols (0, 512, 192, 704)
                            ov = _view(o_ps, 0,
                                       [o_ps.ap[0], [C, 2], [512, 2], [1, C]])
                            obv = ob.rearrange("p (a b c) -> p a b c",
                                               b=2, c=C)
                            if (2 * h + jj) % 2 == 0:
                                nc.scalar.copy(obv, ov)
                            else:
                                nc.vector.tensor_copy(obv, ov)
                            s0 = g * GRP + (h * 2 + jj) * XTB
                            nc.sync.dma_start(
                                out=osw[:, s0 * C:(s0 + XTB) * C], in_=ob)

    nc.compile()
    return nc


_NC = None


def _get_program():
    global _NC
    if _NC is None:
        _NC = build_program()
    return _NC


def _host_prep(Wq, bq, Wk, bk, Wv, bv):
    WqkT = np.concatenate([Wq, Wk], axis=0).T.astype(np.float16)  # [192, 64]
    bqk = np.concatenate([bq, bk], axis=0)[None, :].astype(np.float16)
    wqk1 = np.ascontiguousarray(WqkT[:128])
    wqk2 = np.ascontiguousarray(np.concatenate([WqkT[128:], bqk], axis=0))
    WvT = Wv.T.astype(np.float16)                                 # [192, 192]
    wv1 = np.ascontiguousarray(WvT[:128])
    wv2 = np.ascontiguousarray(
        np.concatenate([WvT[128:], bv[None, :].astype(np.float16)], axis=0))
    return wqk1, wqk2, wv1, wv2


def _make_in_maps(inputs):
    x = np.asarray(inputs["x"], dtype=np.float32)
    x1 = np.asarray(inputs["x1"], dtype=np.float32)
    wqk1, wqk2, wv1, wv2 = _host_prep(
        np.asarray(inputs["Wq"], np.float32), np.asarray(inputs["bq"], np.float32),
        np.asarray(inputs["Wk"], np.float32), np.asarray(inputs["bk"], np.float32),
        np.asarray(inputs["Wv"], np.float32), np.asarray(inputs["bv"], np.float32))
    ident = np.eye(128, dtype=np.float16)
    in_maps = []
    for b in range(B):
        x1h = np.ascontiguousarray(x1[b].reshape(C, N).astype(np.float16))
        xtT = np.ascontiguousarray(
            x[b].reshape(C, N).T.astype(np.float16)
            .reshape(NSUB, 128, C).transpose(1, 0, 2).reshape(128, NSUB * C))
        in_maps.append({
            "x1": x1h, "xt": xtT,
            "wqk1": wqk1, "wqk2": wqk2, "wv1": wv1, "wv2": wv2,
            "ident_d": ident,
        })
    return in_maps


def _unswizzle(osw, gamma):
    # osw [128, NSUB*C] fp16, [p, s*C + c] = out[c, s*128+p] / gamma
    o = np.asarray(osw, np.float32).reshape(128, NSUB, C).transpose(2, 1, 0)
    return (gamma * o.reshape(C, N)).reshape(C, H, W)


def kernel(x, x1, Wq, bq, Wk, bk, Wv, bv, gamma):
    nc = _get_program()
    in_maps = _make_in_maps({
        "x": x, "x1": x1, "Wq": Wq, "bq": bq, "Wk": Wk, "bk": bk,
        "Wv": Wv, "bv": bv})
    res = run_bass_kernel_spmd(nc, in_maps, list(range(N_CORES)))
    g = float(np.asarray(gamma, np.float32).reshape(-1)[0])
    outs = [_unswizzle(res.results[b]["osw"], g) for b in range(B)]
    return np.stack(outs, axis=0).astype(np.float32)
